# revision 5
# baseline (speedup 1.0000x reference)
"""Causal multi-head attention (B=2, L=2048, D=2048, H=32) on 8 trn2 NeuronCores.

Sharding: data-parallel over batch (2 groups of 4 cores) x tensor-parallel over
heads (8 heads per core). Each core computes, for its batch b and head range:
  qhT/khT = (W [dh,D]) @ x.T  (head dims on partitions, tokens on free axis)
  vh      = x @ W.T           (tokens on partitions: natural layout)
  S.T[k,q] = khT.T-block matmuls (contraction over head dim, K=64)
  P.T = exp(S.T) with causal masking (host-precomputed 128x128 triangle mask)
  o.T[d,q] accumulated over k-chunks; softmax denominator rides along as an
  appended ones-column of V (even heads) or a separate M=1 matmul (odd heads)
  normalize via PE broadcast of 1/denom, then out.T = Wo_shard.T.T @ o.T
Host sums the 4 tensor-parallel partials per batch.

All matmuls contract over the partition axis, so the host pre-transposes
q/k/v (free on host, avoids all on-device transposes). Everything is fp32.
"""

import sys

sys.path.insert(0, "/opt/trn_rl_repo")

import numpy as np

import concourse.bass as bass
import concourse.tile as tile
from concourse import bacc, mybir
from concourse.bass_utils import run_bass_kernel_spmd


def _ensure_ntff_hook():
    """The agent image's antenv package lacks axon_hooks, which makes
    run_bass_kernel_spmd(trace=True) crash on import. Provide the module and
    register the ctypes-based NTFF profiling hook (degrades silently)."""
    try:
        import types

        import antenv

        if "antenv.axon_hooks" not in sys.modules:
            m = types.ModuleType("antenv.axon_hooks")
            state = {"hook": None}
            m.set_axon_ntff_profile_hook = lambda h: state.__setitem__("hook", h)
            m.get_axon_ntff_profile_hook = lambda: state["hook"]
            sys.modules["antenv.axon_hooks"] = m
            antenv.axon_hooks = m
        from antenv.axon_hooks import (
            get_axon_ntff_profile_hook,
            set_axon_ntff_profile_hook,
        )

        if get_axon_ntff_profile_hook() is None:
            from trn_agent_boot.trn_boot import _ntff_profile_via_ctypes

            set_axon_ntff_profile_hook(
                _ntff_profile_via_ctypes("/opt/axon/libaxon_pjrt.so")
            )
    except Exception:
        pass


_ensure_ntff_hook()

F32 = mybir.dt.float32

B, L, D, H = 2, 2048, 2048, 32
HD = 64          # head dim
N_CORES = 8
TP = 4           # tensor-parallel width (heads split 4 ways)
HPC = H // TP    # heads per core = 8
DH = HPC * HD    # per-core projected width = 512
SCALE = float(HD) ** -0.5

QB = 512         # query-block width for SDPA
XT = 256         # token-tile width for the projection streaming operand


def _emit(nc, L_=L, D_=D):
    KC = D_ // 128          # contraction chunks for the projections
    NQB = L_ // QB          # query blocks
    NXT = L_ // XT          # projection token tiles
    TC = L_ // 128          # 128-token chunks
    MCH = DH // 128         # head-pair chunks = 4

    xq = nc.dram_tensor("xq", [D_, L_], F32, kind="ExternalInput")
    xk = nc.dram_tensor("xk", [D_, L_], F32, kind="ExternalInput")
    xv = nc.dram_tensor("xv", [D_, L_], F32, kind="ExternalInput")
    wq = nc.dram_tensor("wq", [D_, DH], F32, kind="ExternalInput")
    wk = nc.dram_tensor("wk", [D_, DH], F32, kind="ExternalInput")
    wv = nc.dram_tensor("wv", [D_, DH], F32, kind="ExternalInput")
    wo = nc.dram_tensor("wo", [DH, D_], F32, kind="ExternalInput")
    tri = nc.dram_tensor("tri", [128, 128], F32, kind="ExternalInput")
    outT = nc.dram_tensor("outT", [D_, L_], F32, kind="ExternalOutput")

    EXP = mybir.ActivationFunctionType.Exp

    with tile.TileContext(nc) as tc:
        from contextlib import ExitStack

        with ExitStack() as st:
            constp = st.enter_context(tc.tile_pool(name="const", bufs=1))
            tri_sb = constp.tile([128, 128], F32)
            nc.sync.dma_start(tri_sb[:], tri[:])
            ones_sb = constp.tile([128, 64], F32)
            nc.vector.memset(ones_sb[:], 1.0)

            actp = st.enter_context(tc.tile_pool(name="acts", bufs=1))
            qhT = actp.tile([128, MCH, L_], F32)
            khT = actp.tile([128, MCH, L_], F32)
            # vh: per 128-token chunk, 8 heads x (64 v-dims + ones col)
            vh = actp.tile([128, TC, HPC * (HD + 1)], F32)
            nc.vector.memset(vh[:], 1.0)

            # ---- q/k projections: out[dim_chunk, tokens] = w_chunk.T @ xT ----
            for name, xdram, wdram, dst in (("q", xq, wq, qhT), ("k", xk, wk, khT)):
                with (
                    tc.tile_pool(name=f"w{name}", bufs=1) as wp,
                    tc.tile_pool(name=f"x{name}", bufs=2) as xp,
                    tc.tile_pool(name=f"ps{name}", bufs=4, space="PSUM") as pp,
                ):
                    w_sb = wp.tile([128, KC, DH], F32, tag="w")
                    nc.sync.dma_start(
                        w_sb[:], wdram[:].rearrange("(kc p) m -> p kc m", p=128)
                    )
                    for n in range(NXT):
                        x_sb = xp.tile([128, KC, XT], F32, tag="x")
                        nc.sync.dma_start(
                            x_sb[:],
                            xdram[:, n * XT : (n + 1) * XT].rearrange(
                                "(kc p) t -> p kc t", p=128
                            ),
                        )
                        for m in range(MCH):
                            ps = pp.tile([128, XT], F32, tag="ps")
                            for kc in range(KC):
                                nc.tensor.matmul(
                                    ps[:],
                                    w_sb[:, kc, m * 128 : (m + 1) * 128],
                                    x_sb[:, kc, :],
                                    start=(kc == 0),
                                    stop=(kc == KC - 1),
                                )
                            nc.vector.tensor_copy(
                                dst[:, m, n * XT : (n + 1) * XT], ps[:]
                            )

            # ---- v projection: natural layout, x chunk is the stationary op ----
            with (
                tc.tile_pool(name="wvp", bufs=1) as wp,
                tc.tile_pool(name="xvp", bufs=2) as xp,
                tc.tile_pool(name="psv", bufs=4, space="PSUM") as pp,
            ):
                w_sb = wp.tile([128, KC, DH], F32, tag="w")
                nc.sync.dma_start(
                    w_sb[:], wv[:].rearrange("(kc p) m -> p kc m", p=128)
                )
                for n in range(NXT):
                    x_sb = xp.tile([128, KC, XT], F32, tag="x")
                    nc.sync.dma_start(
                        x_sb[:],
                        xv[:, n * XT : (n + 1) * XT].rearrange(
                            "(kc p) t -> p kc t", p=128
                        ),
                    )
                    for tt in range(XT // 128):
                        ps = pp.tile([128, DH], F32, tag="ps")
                        for kc in range(KC):
                            nc.tensor.matmul(
                                ps[:],
                                x_sb[:, kc, tt * 128 : (tt + 1) * 128],
                                w_sb[:, kc, :],
                                start=(kc == 0),
                                stop=(kc == KC - 1),
                            )
                        tci = n * (XT // 128) + tt
                        dst = vh[:, tci, :].rearrange("p (h c) -> p h c", c=HD + 1)
                        nc.vector.tensor_copy(
                            dst[:, :, 0:HD],
                            ps[:].rearrange("p (h d) -> p h d", d=HD),
                        )

            # ---- SDPA + output accumulation ----
            otp = st.enter_context(tc.tile_pool(name="otp", bufs=1))
            oT = otp.tile([128, MCH, L_], F32)
            with (
                tc.tile_pool(name="pp", bufs=18) as ppool,
                tc.tile_pool(name="dsbp", bufs=3) as dsbp,
                tc.tile_pool(name="sps", bufs=2, space="PSUM") as spool,
                tc.tile_pool(name="ops", bufs=2, space="PSUM") as opool,
                tc.tile_pool(name="dps", bufs=2, space="PSUM") as dpool,
                tc.tile_pool(name="bcps", bufs=2, space="PSUM") as bcpool,
            ):
                for h in range(HPC):
                    half = 64 * (h % 2)
                    mch = h // 2
                    vcol = h * (HD + 1)
                    for qb in range(NQB):
                        kcnt = (qb + 1) * (QB // 128)
                        q0 = qb * QB
                        ptiles = []
                        for kc in range(kcnt):
                            dj = kc - qb * (QB // 128)
                            col0 = 128 * dj if dj > 0 else 0
                            s_ps = spool.tile([128, QB], F32, tag="s")
                            nc.tensor.matmul(
                                s_ps[:, col0:QB],
                                khT[half : half + 64, mch, kc * 128 : (kc + 1) * 128],
                                qhT[half : half + 64, mch, q0 + col0 : q0 + QB],
                                start=True,
                                stop=True,
                            )
                            p_sb = ppool.tile([128, QB], F32, tag="p")
                            if col0 > 0:
                                nc.vector.memset(p_sb[:, 0:col0], 0.0)
                            nc.scalar.activation(
                                p_sb[:, col0:QB], s_ps[:, col0:QB], EXP
                            )
                            if dj >= 0:
                                nc.vector.tensor_mul(
                                    p_sb[:, col0 : col0 + 128],
                                    p_sb[:, col0 : col0 + 128],
                                    tri_sb[:],
                                )
                            ptiles.append(p_sb)

                        o_ps = opool.tile([128, QB], F32, tag="o")
                        if h % 2 == 0:
                            for kc in range(kcnt):
                                nc.tensor.matmul(
                                    o_ps[0:65, :],
                                    vh[:, kc, vcol : vcol + HD + 1],
                                    ptiles[kc][:],
                                    start=(kc == 0),
                                    stop=(kc == kcnt - 1),
                                )
                            den_row = o_ps[64:65, :]
                            o_rows = o_ps[0:64, :]
                            db = 64
                        else:
                            den_ps = dpool.tile([1, QB], F32, tag="d")
                            for kc in range(kcnt):
                                nc.tensor.matmul(
                                    o_ps[64:128, :],
                                    vh[:, kc, vcol : vcol + HD],
                                    ptiles[kc][:],
                                    start=(kc == 0),
                                    stop=(kc == kcnt - 1),
                                )
                                nc.tensor.matmul(
                                    den_ps[:],
                                    vh[:, kc, vcol + HD : vcol + HD + 1],
                                    ptiles[kc][:],
                                    start=(kc == 0),
                                    stop=(kc == kcnt - 1),
                                )
                            den_row = den_ps[0:1, :]
                            o_rows = o_ps[64:128, :]
                            db = 0

                        dsb = dsbp.tile([128, QB], F32, tag="dsb")
                        nc.vector.tensor_copy(dsb[db : db + 1, :], den_row)
                        nc.vector.reciprocal(
                            dsb[db : db + 1, :], dsb[db : db + 1, :]
                        )
                        bc_ps = bcpool.tile([128, QB], F32, tag="bc")
                        nc.tensor.matmul(
                            bc_ps[half : half + 64, :],
                            ones_sb[db : db + 1, 0:64],
                            dsb[db : db + 1, :],
                            start=True,
                            stop=True,
                        )
                        # HW allows at most one PSUM input per vector op, so
                        # stage o into SBUF first, then scale by 1/denom.
                        nc.vector.tensor_copy(
                            oT[half : half + 64, mch, q0 : q0 + QB], o_rows
                        )
                        nc.vector.tensor_mul(
                            oT[half : half + 64, mch, q0 : q0 + QB],
                            oT[half : half + 64, mch, q0 : q0 + QB],
                            bc_ps[half : half + 64, :],
                        )

            # ---- output projection: outT[m,n] = wo_chunk.T @ oT ----
            with (
                tc.tile_pool(name="wop", bufs=1) as wop,
                tc.tile_pool(name="fps", bufs=8, space="PSUM") as fpool,
                tc.tile_pool(name="osbp", bufs=3) as osbp,
            ):
                wo_sb = wop.tile([128, MCH, D_], F32)
                nc.sync.dma_start(
                    wo_sb[:], wo[:].rearrange("(kc p) m -> p kc m", p=128)
                )
                for m in range(D_ // 128):
                    pts = []
                    for n in range(NQB):
                        pt = fpool.tile([128, QB], F32, tag="f")
                        pts.append(pt)
                    for kc2 in range(MCH):
                        for n in range(NQB):
                            nc.tensor.matmul(
                                pts[n][:],
                                wo_sb[:, kc2, m * 128 : (m + 1) * 128],
                                oT[:, kc2, n * QB : (n + 1) * QB],
                                start=(kc2 == 0),
                                stop=(kc2 == MCH - 1),
                            )
                    for n in range(NQB):
                        osb = osbp.tile([128, QB], F32, tag="ot")
                        nc.vector.tensor_copy(osb[:], pts[n][:])
                        nc.sync.dma_start(
                            outT[m * 128 : (m + 1) * 128, n * QB : (n + 1) * QB],
                            osb[:],
                        )
    return nc


def build(L_=L, D_=D):
    nc = bacc.Bacc("TRN2", target_bir_lowering=False, debug=False)
    _emit(nc, L_, D_)
    nc.compile()
    return nc


_NC_CACHE = {}


def _get_nc():
    if "nc" not in _NC_CACHE:
        _NC_CACHE["nc"] = build()
    return _NC_CACHE["nc"]


def make_in_maps(q, k, v, Wq, Wk, Wv, Wo):
    tri_m = np.triu(np.ones((128, 128), dtype=np.float32))
    qT = [np.ascontiguousarray(q[b].T) for b in range(B)]
    kT = [np.ascontiguousarray(k[b].T) for b in range(B)]
    vT = [np.ascontiguousarray(v[b].T) for b in range(B)]
    wq_s, wk_s, wv_s, wo_s = [], [], [], []
    for tp in range(TP):
        rows = slice(tp * DH, (tp + 1) * DH)
        wq_s.append(np.ascontiguousarray(Wq[rows].T * SCALE))
        wk_s.append(np.ascontiguousarray(Wk[rows].T))
        wv_s.append(np.ascontiguousarray(Wv[rows].T))
        wo_s.append(np.ascontiguousarray(Wo[:, rows].T))
    in_maps = []
    for c in range(N_CORES):
        b, tp = c // TP, c % TP
        in_maps.append(
            {
                "xq": qT[b],
                "xk": kT[b],
                "xv": vT[b],
                "wq": wq_s[tp],
                "wk": wk_s[tp],
                "wv": wv_s[tp],
                "wo": wo_s[tp],
                "tri": tri_m,
            }
        )
    return in_maps


def kernel(q, k, v, Wq, Wk, Wv, Wo, mask=None, trace=False):
    q = np.asarray(q, dtype=np.float32)
    k = np.asarray(k, dtype=np.float32)
    v = np.asarray(v, dtype=np.float32)
    nc = _get_nc()
    in_maps = make_in_maps(
        q, k, v,
        np.asarray(Wq, np.float32), np.asarray(Wk, np.float32),
        np.asarray(Wv, np.float32), np.asarray(Wo, np.float32),
    )
    res = run_bass_kernel_spmd(
        nc, in_maps, core_ids=list(range(N_CORES)), trace=trace
    )
    out = np.zeros((B, L, D), dtype=np.float32)
    for c in range(N_CORES):
        out[c // TP] += res.results[c]["outT"].T
    if trace:
        return out, res
    return out


# revision 9
# speedup vs baseline: 2.4458x; 2.4458x over previous
"""Causal multi-head attention (B=2, L=2048, D=2048, H=32) on 8 trn2 NeuronCores.

Sharding: data-parallel over batch (2 groups of 4 cores) x tensor-parallel over
heads (8 heads per core). Each core computes, for its batch b and head range:
  qhT/khT = (W [dh,D]) @ x.T  (head dims on partitions, tokens on free axis)
  vh      = x @ W.T           (tokens on partitions: natural layout)
  S.T[k,q] = khT.T-block matmuls (contraction over head dim, K=64)
  P.T = exp(S.T) with causal masking (host-precomputed 128x128 triangle mask)
  o.T[d,q] accumulated over k-chunks; softmax denominator rides along as an
  appended ones-column of V (even heads) or a separate M=1 matmul (odd heads)
  normalize via PE broadcast of 1/denom, then out.T = Wo_shard.T.T @ o.T
Host sums the 4 tensor-parallel partials per batch.

All matmuls contract over the partition axis, so the host pre-transposes
q/k/v (free on host, avoids all on-device transposes). Everything is fp32.
"""

import sys

sys.path.insert(0, "/opt/trn_rl_repo")

import numpy as np

import concourse.bass as bass
import concourse.tile as tile
from concourse import bacc, mybir
from concourse.bass_utils import run_bass_kernel_spmd


def _ensure_ntff_hook():
    """The agent image's antenv package lacks axon_hooks, which makes
    run_bass_kernel_spmd(trace=True) crash on import. Provide the module and
    register the ctypes-based NTFF profiling hook (degrades silently)."""
    try:
        import types

        import antenv

        if "antenv.axon_hooks" not in sys.modules:
            m = types.ModuleType("antenv.axon_hooks")
            state = {"hook": None}
            m.set_axon_ntff_profile_hook = lambda h: state.__setitem__("hook", h)
            m.get_axon_ntff_profile_hook = lambda: state["hook"]
            sys.modules["antenv.axon_hooks"] = m
            antenv.axon_hooks = m
        from antenv.axon_hooks import (
            get_axon_ntff_profile_hook,
            set_axon_ntff_profile_hook,
        )

        if get_axon_ntff_profile_hook() is None:
            from trn_agent_boot.trn_boot import _ntff_profile_via_ctypes

            set_axon_ntff_profile_hook(
                _ntff_profile_via_ctypes("/opt/axon/libaxon_pjrt.so")
            )
    except Exception:
        pass


_ensure_ntff_hook()

F32 = mybir.dt.float32
F32R = mybir.dt.float32r

B, L, D, H = 2, 2048, 2048, 32
HD = 64          # head dim
N_CORES = 8
TP = 4           # tensor-parallel width (heads split 4 ways)
HPC = H // TP    # heads per core = 8
DH = HPC * HD    # per-core projected width = 512
SCALE = float(HD) ** -0.5

QB = 512         # query-block width for SDPA
XT = 256         # token-tile width for the projection streaming operand


def _emit(nc, L_=L, D_=D):
    KC = D_ // 128          # contraction chunks for the projections
    NQB = L_ // QB          # query blocks
    NXT = L_ // XT          # projection token tiles
    TC = L_ // 128          # 128-token chunks
    MCH = DH // 128         # head-pair chunks = 4

    xq = nc.dram_tensor("xq", [D_, L_], F32R, kind="ExternalInput")
    xk = nc.dram_tensor("xk", [D_, L_], F32R, kind="ExternalInput")
    xv = nc.dram_tensor("xv", [D_, L_], F32R, kind="ExternalInput")
    wq = nc.dram_tensor("wq", [D_, DH], F32R, kind="ExternalInput")
    wk = nc.dram_tensor("wk", [D_, DH], F32R, kind="ExternalInput")
    wv = nc.dram_tensor("wv", [D_, DH], F32R, kind="ExternalInput")
    wo = nc.dram_tensor("wo", [DH, D_], F32R, kind="ExternalInput")
    konst = nc.dram_tensor("konst", [128, 640], F32R, kind="ExternalInput")
    outT = nc.dram_tensor("outT", [D_, L_], F32, kind="ExternalOutput")

    EXP = mybir.ActivationFunctionType.Exp

    with tile.TileContext(nc) as tc:
        from contextlib import ExitStack

        with ExitStack() as st:
            constp = st.enter_context(tc.tile_pool(name="const", bufs=1))
            ksb = constp.tile([128, 640], F32R)
            nc.sync.dma_start(ksb[:], konst[:])
            tri_sb = ksb[:, 0:128]
            ones_sb = constp.tile([128, 64], F32)
            nc.vector.memset(ones_sb[:], 1.0)

            actp = st.enter_context(tc.tile_pool(name="acts", bufs=1))
            qhT = actp.tile([128, MCH, L_], F32R)
            khT = actp.tile([128, MCH, L_], F32R)
            # vh: per 128-token chunk, 8 heads x (64 v-dims + ones col)
            vh = actp.tile([128, TC, HPC * (HD + 1)], F32R)
            # ones columns (softmax denominator trick): copy from konst block
            vh_r = vh[:, :, :].rearrange("p t (h c) -> p t h c", c=HD + 1)
            nc.vector.tensor_copy(
                vh_r[:, :, :, HD : HD + 1],
                ksb[:, 128 : 128 + TC * HPC].rearrange(
                    "p (t h one) -> p t h one", h=HPC, one=1
                ),
            )

            # ---- q/k projections: out[dim_chunk, tokens] = w_chunk.T @ xT ----
            for name, xdram, wdram, dst in (("q", xq, wq, qhT), ("k", xk, wk, khT)):
                with (
                    tc.tile_pool(name=f"w{name}", bufs=1) as wp,
                    tc.tile_pool(name=f"x{name}", bufs=2) as xp,
                    tc.tile_pool(name=f"ps{name}", bufs=4, space="PSUM") as pp,
                ):
                    w_sb = wp.tile([128, KC, DH], F32R, tag="w")
                    nc.sync.dma_start(
                        w_sb[:], wdram[:].rearrange("(kc p) m -> p kc m", p=128)
                    )
                    for n in range(NXT):
                        x_sb = xp.tile([128, KC, XT], F32R, tag="x")
                        nc.sync.dma_start(
                            x_sb[:],
                            xdram[:, n * XT : (n + 1) * XT].rearrange(
                                "(kc p) t -> p kc t", p=128
                            ),
                        )
                        for m in range(MCH):
                            ps = pp.tile([128, XT], F32, tag="ps")
                            for kc in range(KC):
                                nc.tensor.matmul(
                                    ps[:],
                                    w_sb[:, kc, m * 128 : (m + 1) * 128],
                                    x_sb[:, kc, :],
                                    start=(kc == 0),
                                    stop=(kc == KC - 1),
                                )
                            nc.vector.tensor_copy(
                                dst[:, m, n * XT : (n + 1) * XT], ps[:]
                            )

            # ---- v projection: natural layout, x chunk is the stationary op ----
            with (
                tc.tile_pool(name="wvp", bufs=1) as wp,
                tc.tile_pool(name="xvp", bufs=2) as xp,
                tc.tile_pool(name="psv", bufs=4, space="PSUM") as pp,
            ):
                w_sb = wp.tile([128, KC, DH], F32R, tag="w")
                nc.sync.dma_start(
                    w_sb[:], wv[:].rearrange("(kc p) m -> p kc m", p=128)
                )
                for n in range(NXT):
                    x_sb = xp.tile([128, KC, XT], F32R, tag="x")
                    nc.sync.dma_start(
                        x_sb[:],
                        xv[:, n * XT : (n + 1) * XT].rearrange(
                            "(kc p) t -> p kc t", p=128
                        ),
                    )
                    for tt in range(XT // 128):
                        ps = pp.tile([128, DH], F32, tag="ps")
                        for kc in range(KC):
                            nc.tensor.matmul(
                                ps[:],
                                x_sb[:, kc, tt * 128 : (tt + 1) * 128],
                                w_sb[:, kc, :],
                                start=(kc == 0),
                                stop=(kc == KC - 1),
                            )
                        tci = n * (XT // 128) + tt
                        dst = vh[:, tci, :].rearrange("p (h c) -> p h c", c=HD + 1)
                        nc.vector.tensor_copy(
                            dst[:, :, 0:HD],
                            ps[:].rearrange("p (h d) -> p h d", d=HD),
                        )

            # ---- SDPA + output accumulation ----
            otp = st.enter_context(tc.tile_pool(name="otp", bufs=1))
            oT = otp.tile([128, MCH, L_], F32R)
            with (
                tc.tile_pool(name="pp", bufs=18) as ppool,
                tc.tile_pool(name="dsbp", bufs=3) as dsbp,
                tc.tile_pool(name="stgp", bufs=3) as stgp,
                tc.tile_pool(name="sps", bufs=3, space="PSUM") as spool,
                tc.tile_pool(name="ops", bufs=3, space="PSUM") as opool,
                tc.tile_pool(name="bcps", bufs=2, space="PSUM") as bcpool,
            ):
                for h in range(HPC):
                    half = 64 * (h % 2)
                    mch = h // 2
                    vcol = h * (HD + 1)
                    for qb in range(NQB):
                        kcnt = (qb + 1) * (QB // 128)
                        q0 = qb * QB
                        ptiles = []
                        for kc in range(kcnt):
                            dj = kc - qb * (QB // 128)
                            col0 = 128 * dj if dj > 0 else 0
                            s_ps = spool.tile([128, QB], F32, tag="s")
                            nc.tensor.matmul(
                                s_ps[:, col0:QB],
                                khT[half : half + 64, mch, kc * 128 : (kc + 1) * 128],
                                qhT[half : half + 64, mch, q0 + col0 : q0 + QB],
                                start=True,
                                stop=True,
                            )
                            p_sb = ppool.tile([128, QB], F32R, tag="p")
                            if col0 > 0:
                                nc.vector.tensor_copy(
                                    p_sb[:, 0:col0], ksb[:, 256 : 256 + col0]
                                )
                            nc.scalar.activation(
                                p_sb[:, col0:QB], s_ps[:, col0:QB], EXP
                            )
                            if dj >= 0:
                                nc.vector.tensor_mul(
                                    p_sb[:, col0 : col0 + 128],
                                    p_sb[:, col0 : col0 + 128],
                                    tri_sb[:],
                                )
                            ptiles.append(p_sb)

                        # One accumulation per head at psum base 0: 64 o-rows
                        # plus the denominator row from the ones-column of vh.
                        # (f32r matmuls reject a column tile_position, so odd
                        # heads can't target psum rows 64-127 directly; they
                        # stage in SBUF and DMA to oT's upper partitions.)
                        o_ps = opool.tile([128, QB], F32, tag="o")
                        for kc in range(kcnt):
                            nc.tensor.matmul(
                                o_ps[0:65, :],
                                vh[:, kc, vcol : vcol + HD + 1],
                                ptiles[kc][:],
                                start=(kc == 0),
                                stop=(kc == kcnt - 1),
                            )

                        dsb = dsbp.tile([128, QB], F32, tag="dsb")
                        nc.vector.tensor_copy(dsb[64:65, :], o_ps[64:65, :])
                        nc.vector.reciprocal(dsb[64:65, :], dsb[64:65, :])
                        bc_ps = bcpool.tile([128, QB], F32, tag="bc")
                        nc.tensor.matmul(
                            bc_ps[0:64, :],
                            ones_sb[64:65, 0:64],
                            dsb[64:65, :],
                            start=True,
                            stop=True,
                        )
                        # At most one PSUM input per vector op: stage o into
                        # SBUF first, then scale by 1/denom.
                        if h % 2 == 0:
                            dst = oT[0:64, mch, q0 : q0 + QB]
                            nc.vector.tensor_copy(dst, o_ps[0:64, :])
                            nc.vector.tensor_mul(dst, dst, bc_ps[0:64, :])
                        else:
                            stg = stgp.tile([64, QB], F32R, tag="stg")
                            nc.vector.tensor_copy(stg[:], o_ps[0:64, :])
                            nc.vector.tensor_mul(stg[:], stg[:], bc_ps[0:64, :])
                            nc.sync.dma_start(
                                oT[64:128, mch, q0 : q0 + QB], stg[:]
                            )

            # ---- output projection: outT[m,n] = wo_chunk.T @ oT ----
            with (
                tc.tile_pool(name="wop", bufs=1) as wop,
                tc.tile_pool(name="fps", bufs=8, space="PSUM") as fpool,
                tc.tile_pool(name="osbp", bufs=3) as osbp,
            ):
                wo_sb = wop.tile([128, MCH, D_], F32R)
                nc.sync.dma_start(
                    wo_sb[:], wo[:].rearrange("(kc p) m -> p kc m", p=128)
                )
                for m in range(D_ // 128):
                    pts = []
                    for n in range(NQB):
                        pt = fpool.tile([128, QB], F32, tag="f")
                        pts.append(pt)
                    for kc2 in range(MCH):
                        for n in range(NQB):
                            nc.tensor.matmul(
                                pts[n][:],
                                wo_sb[:, kc2, m * 128 : (m + 1) * 128],
                                oT[:, kc2, n * QB : (n + 1) * QB],
                                start=(kc2 == 0),
                                stop=(kc2 == MCH - 1),
                            )
                    for n in range(NQB):
                        osb = osbp.tile([128, QB], F32, tag="ot")
                        nc.vector.tensor_copy(osb[:], pts[n][:])
                        nc.sync.dma_start(
                            outT[m * 128 : (m + 1) * 128, n * QB : (n + 1) * QB],
                            osb[:],
                        )
    return nc


def build(L_=L, D_=D):
    nc = bacc.Bacc("TRN2", target_bir_lowering=False, debug=False)
    _emit(nc, L_, D_)
    nc.compile()
    return nc


_NC_CACHE = {}


def _get_nc():
    if "nc" not in _NC_CACHE:
        _NC_CACHE["nc"] = build()
    return _NC_CACHE["nc"]


def make_in_maps(q, k, v, Wq, Wk, Wv, Wo):
    konst_m = np.zeros((128, 640), dtype=np.float32)
    konst_m[:, 0:128] = np.triu(np.ones((128, 128), dtype=np.float32))
    konst_m[:, 128:256] = 1.0
    qT = [np.ascontiguousarray(q[b].T) for b in range(B)]
    kT = [np.ascontiguousarray(k[b].T) for b in range(B)]
    vT = [np.ascontiguousarray(v[b].T) for b in range(B)]
    wq_s, wk_s, wv_s, wo_s = [], [], [], []
    for tp in range(TP):
        rows = slice(tp * DH, (tp + 1) * DH)
        wq_s.append(np.ascontiguousarray(Wq[rows].T * SCALE))
        wk_s.append(np.ascontiguousarray(Wk[rows].T))
        wv_s.append(np.ascontiguousarray(Wv[rows].T))
        wo_s.append(np.ascontiguousarray(Wo[:, rows].T))
    in_maps = []
    for c in range(N_CORES):
        b, tp = c // TP, c % TP
        in_maps.append(
            {
                "xq": qT[b],
                "xk": kT[b],
                "xv": vT[b],
                "wq": wq_s[tp],
                "wk": wk_s[tp],
                "wv": wv_s[tp],
                "wo": wo_s[tp],
                "konst": konst_m,
            }
        )
    return in_maps


def kernel(q, k, v, Wq, Wk, Wv, Wo, mask=None, trace=False):
    q = np.asarray(q, dtype=np.float32)
    k = np.asarray(k, dtype=np.float32)
    v = np.asarray(v, dtype=np.float32)
    nc = _get_nc()
    in_maps = make_in_maps(
        q, k, v,
        np.asarray(Wq, np.float32), np.asarray(Wk, np.float32),
        np.asarray(Wv, np.float32), np.asarray(Wo, np.float32),
    )
    res = run_bass_kernel_spmd(
        nc, in_maps, core_ids=list(range(N_CORES)), trace=trace
    )
    out = np.zeros((B, L, D), dtype=np.float32)
    for c in range(N_CORES):
        out[c // TP] += res.results[c]["outT"].T
    if trace:
        return out, res
    return out


# revision 11
# speedup vs baseline: 2.5287x; 1.0339x over previous
"""Causal multi-head attention (B=2, L=2048, D=2048, H=32) on 8 trn2 NeuronCores.

Sharding: data-parallel over batch (2 groups of 4 cores) x tensor-parallel over
heads (8 heads per core). Each core computes, for its batch b and head range:
  qhT/khT = (W [dh,D]) @ x.T  (head dims on partitions, tokens on free axis)
  vh      = x @ W.T           (tokens on partitions: natural layout)
  S.T[k,q] = khT.T-block matmuls (contraction over head dim, K=64)
  P.T = exp(S.T) with causal masking (host-precomputed 128x128 triangle mask)
  o.T[d,q] accumulated over k-chunks; softmax denominator rides along as an
  appended ones-column of V (even heads) or a separate M=1 matmul (odd heads)
  normalize via PE broadcast of 1/denom, then out.T = Wo_shard.T.T @ o.T
Host sums the 4 tensor-parallel partials per batch.

All matmuls contract over the partition axis, so the host pre-transposes
q/k/v (free on host, avoids all on-device transposes). Everything is fp32.
"""

import sys

sys.path.insert(0, "/opt/trn_rl_repo")

import numpy as np

import concourse.bass as bass
import concourse.tile as tile
from concourse import bacc, mybir
from concourse.bass_utils import run_bass_kernel_spmd


def _ensure_ntff_hook():
    """The agent image's antenv package lacks axon_hooks, which makes
    run_bass_kernel_spmd(trace=True) crash on import. Provide the module and
    register the ctypes-based NTFF profiling hook (degrades silently)."""
    try:
        import types

        import antenv

        if "antenv.axon_hooks" not in sys.modules:
            m = types.ModuleType("antenv.axon_hooks")
            state = {"hook": None}
            m.set_axon_ntff_profile_hook = lambda h: state.__setitem__("hook", h)
            m.get_axon_ntff_profile_hook = lambda: state["hook"]
            sys.modules["antenv.axon_hooks"] = m
            antenv.axon_hooks = m
        from antenv.axon_hooks import (
            get_axon_ntff_profile_hook,
            set_axon_ntff_profile_hook,
        )

        if get_axon_ntff_profile_hook() is None:
            from trn_agent_boot.trn_boot import _ntff_profile_via_ctypes

            set_axon_ntff_profile_hook(
                _ntff_profile_via_ctypes("/opt/axon/libaxon_pjrt.so")
            )
    except Exception:
        pass


_ensure_ntff_hook()

F32 = mybir.dt.float32
F32R = mybir.dt.float32r

B, L, D, H = 2, 2048, 2048, 32
HD = 64          # head dim
N_CORES = 8
TP = 4           # tensor-parallel width (heads split 4 ways)
HPC = H // TP    # heads per core = 8
DH = HPC * HD    # per-core projected width = 512
SCALE = float(HD) ** -0.5

QB = 512         # query-block width for SDPA
XT = 256         # token-tile width for the projection streaming operand


def _emit(nc, L_=L, D_=D):
    KC = D_ // 128          # contraction chunks for the projections
    NQB = L_ // QB          # query blocks
    NXT = L_ // XT          # projection token tiles
    TC = L_ // 128          # 128-token chunks
    MCH = DH // 128         # head-pair chunks = 4

    xq = nc.dram_tensor("xq", [D_, L_], F32R, kind="ExternalInput")
    xk = nc.dram_tensor("xk", [D_, L_], F32R, kind="ExternalInput")
    xv = nc.dram_tensor("xv", [D_, L_], F32R, kind="ExternalInput")
    wq = nc.dram_tensor("wq", [D_, DH], F32R, kind="ExternalInput")
    wk = nc.dram_tensor("wk", [D_, DH], F32R, kind="ExternalInput")
    wv = nc.dram_tensor("wv", [D_, DH], F32R, kind="ExternalInput")
    wo = nc.dram_tensor("wo", [DH, D_], F32R, kind="ExternalInput")
    konst = nc.dram_tensor("konst", [128, 640], F32R, kind="ExternalInput")
    outT = nc.dram_tensor("outT", [D_, L_], F32, kind="ExternalOutput")

    EXP = mybir.ActivationFunctionType.Exp

    with tile.TileContext(nc) as tc:
        from contextlib import ExitStack

        with ExitStack() as st:
            constp = st.enter_context(tc.tile_pool(name="const", bufs=1))
            ksb = constp.tile([128, 640], F32R)
            nc.sync.dma_start(ksb[:], konst[:])
            tri_sb = ksb[:, 0:128]
            ones_sb = constp.tile([128, 64], F32)
            nc.vector.memset(ones_sb[:], 1.0)

            actp = st.enter_context(tc.tile_pool(name="acts", bufs=1))
            qhT = actp.tile([128, MCH, L_], F32R)
            khT = actp.tile([128, MCH, L_], F32R)
            # vh: per 128-token chunk, 8 heads x (64 v-dims + ones col)
            vh = actp.tile([128, TC, HPC * (HD + 1)], F32R)
            # ones columns (softmax denominator trick): copy from konst block
            vh_r = vh[:, :, :].rearrange("p t (h c) -> p t h c", c=HD + 1)
            nc.vector.tensor_copy(
                vh_r[:, :, :, HD : HD + 1],
                ksb[:, 128 : 128 + TC * HPC].rearrange(
                    "p (t h one) -> p t h one", h=HPC, one=1
                ),
            )

            # ---- q/k projections: out[dim_chunk, tokens] = w_chunk.T @ xT ----
            for name, xdram, wdram, dst in (("q", xq, wq, qhT), ("k", xk, wk, khT)):
                with (
                    tc.tile_pool(name=f"w{name}", bufs=1) as wp,
                    tc.tile_pool(name=f"x{name}", bufs=2) as xp,
                    tc.tile_pool(name=f"ps{name}", bufs=4, space="PSUM") as pp,
                ):
                    w_sb = wp.tile([128, KC, DH], F32R, tag="w")
                    nc.sync.dma_start(
                        w_sb[:], wdram[:].rearrange("(kc p) m -> p kc m", p=128)
                    )
                    for n in range(NXT):
                        x_sb = xp.tile([128, KC, XT], F32R, tag="x")
                        nc.sync.dma_start(
                            x_sb[:],
                            xdram[:, n * XT : (n + 1) * XT].rearrange(
                                "(kc p) t -> p kc t", p=128
                            ),
                        )
                        for m in range(MCH):
                            ps = pp.tile([128, XT], F32, tag="ps")
                            for kc in range(KC):
                                nc.tensor.matmul(
                                    ps[:],
                                    w_sb[:, kc, m * 128 : (m + 1) * 128],
                                    x_sb[:, kc, :],
                                    start=(kc == 0),
                                    stop=(kc == KC - 1),
                                )
                            nc.vector.tensor_copy(
                                dst[:, m, n * XT : (n + 1) * XT], ps[:]
                            )

            # ---- v projection: natural layout, x chunk is the stationary op ----
            with (
                tc.tile_pool(name="wvp", bufs=1) as wp,
                tc.tile_pool(name="xvp", bufs=2) as xp,
                tc.tile_pool(name="psv", bufs=4, space="PSUM") as pp,
            ):
                w_sb = wp.tile([128, KC, DH], F32R, tag="w")
                nc.sync.dma_start(
                    w_sb[:], wv[:].rearrange("(kc p) m -> p kc m", p=128)
                )
                for n in range(NXT):
                    x_sb = xp.tile([128, KC, XT], F32R, tag="x")
                    nc.sync.dma_start(
                        x_sb[:],
                        xv[:, n * XT : (n + 1) * XT].rearrange(
                            "(kc p) t -> p kc t", p=128
                        ),
                    )
                    for tt in range(XT // 128):
                        ps = pp.tile([128, DH], F32, tag="ps")
                        for kc in range(KC):
                            nc.tensor.matmul(
                                ps[:],
                                x_sb[:, kc, tt * 128 : (tt + 1) * 128],
                                w_sb[:, kc, :],
                                start=(kc == 0),
                                stop=(kc == KC - 1),
                            )
                        tci = n * (XT // 128) + tt
                        dst = vh[:, tci, :].rearrange("p (h c) -> p h c", c=HD + 1)
                        nc.vector.tensor_copy(
                            dst[:, :, 0:HD],
                            ps[:].rearrange("p (h d) -> p h d", d=HD),
                        )

            # ---- SDPA + output accumulation ----
            otp = st.enter_context(tc.tile_pool(name="otp", bufs=1))
            oT = otp.tile([128, MCH, L_], F32R)
            with (
                tc.tile_pool(name="pp", bufs=20) as ppool,
                tc.tile_pool(name="dsbp", bufs=3) as dsbp,
                tc.tile_pool(name="stgp", bufs=3) as stgp,
                tc.tile_pool(name="sps", bufs=3, space="PSUM") as spool,
                tc.tile_pool(name="ops", bufs=3, space="PSUM") as opool,
                tc.tile_pool(name="bcps", bufs=2, space="PSUM") as bcpool,
            ):
                # Software pipeline over (head, q-block) blocks so the PE
                # never stalls on the ACT exp latency: block B's o-matmuls
                # are interleaved with block B+1's score matmuls, and the
                # normalization (which waits on a DVE reciprocal) trails by
                # two blocks.
                blocks = [(h, qb) for h in range(HPC) for qb in range(NQB)]
                KPQ = QB // 128
                state = {}

                def kcnt_of(b):
                    return (b[1] + 1) * KPQ

                def emit_s_step(b, kc):
                    h, qb = b
                    half, mch = 64 * (h % 2), h // 2
                    q0 = qb * QB
                    dj = kc - qb * KPQ
                    col0 = 128 * dj if dj > 0 else 0
                    s_ps = spool.tile([128, QB], F32, tag="s", name="s_ps")
                    nc.tensor.matmul(
                        s_ps[:, col0:QB],
                        khT[half : half + 64, mch, kc * 128 : (kc + 1) * 128],
                        qhT[half : half + 64, mch, q0 + col0 : q0 + QB],
                        start=True,
                        stop=True,
                    )
                    p_sb = ppool.tile([128, QB], F32R, tag="p", name="p_sb")
                    if col0 > 0:
                        nc.vector.tensor_copy(
                            p_sb[:, 0:col0], ksb[:, 256 : 256 + col0]
                        )
                    nc.scalar.activation(p_sb[:, col0:QB], s_ps[:, col0:QB], EXP)
                    if dj >= 0:
                        nc.vector.tensor_mul(
                            p_sb[:, col0 : col0 + 128],
                            p_sb[:, col0 : col0 + 128],
                            tri_sb[:],
                        )
                    state[b]["p"].append(p_sb)

                def emit_o_step(b, kc):
                    # One accumulation per head at psum base 0: 64 o-rows plus
                    # the denominator row from the ones-column of vh. (f32r
                    # matmuls reject a column tile_position, so odd heads
                    # can't target psum rows 64-127 directly; they stage in
                    # SBUF and DMA into oT's upper partitions.)
                    h, qb = b
                    st_ = state[b]
                    if kc == 0:
                        st_["o"] = opool.tile([128, QB], F32, tag="o", name="o_ps")
                    nc.tensor.matmul(
                        st_["o"][0:65, :],
                        vh[:, kc, h * (HD + 1) : h * (HD + 1) + HD + 1],
                        st_["p"][kc][:],
                        start=(kc == 0),
                        stop=(kc == kcnt_of(b) - 1),
                    )

                def emit_recip(b):
                    st_ = state[b]
                    dsb = dsbp.tile([65, 2 * QB], F32, tag="dsb", name="dsb")
                    nc.vector.tensor_copy(dsb[64:65, 0:QB], st_["o"][64:65, :])
                    nc.vector.reciprocal(
                        dsb[64:65, QB : 2 * QB], dsb[64:65, 0:QB]
                    )
                    st_["dsb"] = dsb

                def emit_norm(b):
                    h, qb = b
                    mch, q0 = h // 2, qb * QB
                    st_ = state.pop(b)
                    bc_ps = bcpool.tile([128, QB], F32, tag="bc", name="bc_ps")
                    nc.tensor.matmul(
                        bc_ps[0:64, :],
                        ones_sb[64:65, 0:64],
                        st_["dsb"][64:65, QB : 2 * QB],
                        start=True,
                        stop=True,
                    )
                    # At most one PSUM input per vector op: stage o into SBUF
                    # first, then scale by 1/denom.
                    if h % 2 == 0:
                        dst = oT[0:64, mch, q0 : q0 + QB]
                        nc.vector.tensor_copy(dst, st_["o"][0:64, :])
                        nc.vector.tensor_mul(dst, dst, bc_ps[0:64, :])
                    else:
                        stg = stgp.tile([64, QB], F32R, tag="stg", name="stg")
                        nc.vector.tensor_copy(stg[:], st_["o"][0:64, :])
                        nc.vector.tensor_mul(stg[:], stg[:], bc_ps[0:64, :])
                        nc.sync.dma_start(oT[64:128, mch, q0 : q0 + QB], stg[:])

                seq = blocks + [None, None]
                for idx, b in enumerate(seq):
                    prev = seq[idx - 1] if idx >= 1 else None
                    prev2 = seq[idx - 2] if idx >= 2 else None
                    if b is not None:
                        state[b] = {"p": []}
                    ns = kcnt_of(b) if b is not None else 0
                    no = kcnt_of(prev) if prev is not None else 0
                    for i in range(max(ns, no)):
                        if i < ns:
                            emit_s_step(b, i)
                        if i < no:
                            emit_o_step(prev, i)
                    if prev is not None:
                        emit_recip(prev)
                    if prev2 is not None:
                        emit_norm(prev2)

            # ---- output projection: outT[m,n] = wo_chunk.T @ oT ----
            with (
                tc.tile_pool(name="wop", bufs=1) as wop,
                tc.tile_pool(name="fps", bufs=8, space="PSUM") as fpool,
                tc.tile_pool(name="osbp", bufs=3) as osbp,
            ):
                wo_sb = wop.tile([128, MCH, D_], F32R)
                nc.sync.dma_start(
                    wo_sb[:], wo[:].rearrange("(kc p) m -> p kc m", p=128)
                )
                for m in range(D_ // 128):
                    pts = []
                    for n in range(NQB):
                        pt = fpool.tile([128, QB], F32, tag="f")
                        pts.append(pt)
                    for kc2 in range(MCH):
                        for n in range(NQB):
                            nc.tensor.matmul(
                                pts[n][:],
                                wo_sb[:, kc2, m * 128 : (m + 1) * 128],
                                oT[:, kc2, n * QB : (n + 1) * QB],
                                start=(kc2 == 0),
                                stop=(kc2 == MCH - 1),
                            )
                    for n in range(NQB):
                        osb = osbp.tile([128, QB], F32, tag="ot")
                        nc.vector.tensor_copy(osb[:], pts[n][:])
                        nc.sync.dma_start(
                            outT[m * 128 : (m + 1) * 128, n * QB : (n + 1) * QB],
                            osb[:],
                        )
    return nc


def build(L_=L, D_=D):
    nc = bacc.Bacc("TRN2", target_bir_lowering=False, debug=False)
    _emit(nc, L_, D_)
    nc.compile()
    return nc


_NC_CACHE = {}


def _get_nc():
    if "nc" not in _NC_CACHE:
        _NC_CACHE["nc"] = build()
    return _NC_CACHE["nc"]


def make_in_maps(q, k, v, Wq, Wk, Wv, Wo):
    konst_m = np.zeros((128, 640), dtype=np.float32)
    konst_m[:, 0:128] = np.triu(np.ones((128, 128), dtype=np.float32))
    konst_m[:, 128:256] = 1.0
    qT = [np.ascontiguousarray(q[b].T) for b in range(B)]
    kT = [np.ascontiguousarray(k[b].T) for b in range(B)]
    vT = [np.ascontiguousarray(v[b].T) for b in range(B)]
    wq_s, wk_s, wv_s, wo_s = [], [], [], []
    for tp in range(TP):
        rows = slice(tp * DH, (tp + 1) * DH)
        wq_s.append(np.ascontiguousarray(Wq[rows].T * SCALE))
        wk_s.append(np.ascontiguousarray(Wk[rows].T))
        wv_s.append(np.ascontiguousarray(Wv[rows].T))
        wo_s.append(np.ascontiguousarray(Wo[:, rows].T))
    in_maps = []
    for c in range(N_CORES):
        b, tp = c // TP, c % TP
        in_maps.append(
            {
                "xq": qT[b],
                "xk": kT[b],
                "xv": vT[b],
                "wq": wq_s[tp],
                "wk": wk_s[tp],
                "wv": wv_s[tp],
                "wo": wo_s[tp],
                "konst": konst_m,
            }
        )
    return in_maps


def kernel(q, k, v, Wq, Wk, Wv, Wo, mask=None, trace=False):
    q = np.asarray(q, dtype=np.float32)
    k = np.asarray(k, dtype=np.float32)
    v = np.asarray(v, dtype=np.float32)
    nc = _get_nc()
    in_maps = make_in_maps(
        q, k, v,
        np.asarray(Wq, np.float32), np.asarray(Wk, np.float32),
        np.asarray(Wv, np.float32), np.asarray(Wo, np.float32),
    )
    res = run_bass_kernel_spmd(
        nc, in_maps, core_ids=list(range(N_CORES)), trace=trace
    )
    out = np.zeros((B, L, D), dtype=np.float32)
    for c in range(N_CORES):
        out[c // TP] += res.results[c]["outT"].T
    if trace:
        return out, res
    return out
